# revision 8
# baseline (speedup 1.0000x reference)
"""ConvDeepSet kernel for Trainium2 (8 NeuronCores, batch-parallel).

Reference computation (per batch b):
    dists[n,m] = (x[n,0]-t[m,0])^2 + (x[n,1]-t[m,1])^2
    wt_c[n,m]  = exp(-0.5 * dists / s_c^2),  s = exp(sigma)
    dens[m]    = sum_n wt_0[n,m]
    conv[m]    = sum_n y[n] * wt_1[n,m]
    feat[m]    = [dens, conv/(dens+1e-8)]
    out[m,o]   = feat[m] @ W[o,:]^T + b[o]

Device mapping (one batch per core), all-bf16 matmul operands:
  - dist[n,m] = |x_n|^2 + |t_m|^2 - 2 x_n.t_m as a K=24 augmented matmul on
    the TensorEngine.  Host splits the fp64 augmented operands into three
    bf16 levels each; the 6 cross terms with i+j<=2 reproduce dist to
    ~1e-5 absolute (validated: end-to-end rel err 2.7e-3 vs 2e-2 budget).
    bf16 weights get fast (FWL) background weight loads -- fp32/f32r weights
    serialize a 280-330ns LDWEIGHTS before every matmul, which dominated the
    tensor-engine time in the fp32r version of this kernel.
  - wt = exp(scale * dist) on the ScalarEngine (PSUM -> SBUF, bf16 out).
  - [dens; conv] via K=128 reduce-matmul: lhsT = [1, y] (128 x 2 bf16),
    rhs = wt, accumulated over the 8 n-tiles in PSUM.
  - conv/(dens+eps) on the VectorEngine after a DMA repack to [128, x].
  - final projection as a K=3 bf16 matmul: lhsT = [dens; conv/dens; 1],
    rhs = [W[:,0]; W[:,1]; b] (3 x 64) -> out tile [128, 64] fp32.
"""

import numpy as np
import ml_dtypes

BF16 = ml_dtypes.bfloat16

B = 8
N_IN = 1024
N_OUT = 4096
OUT_CH = 64
P = 128
NT = N_IN // P  # 8 n-tiles
CHUNK = 1024  # m-chunk (free size of one dist PSUM tile / one bf16 matmul)
NCH = N_OUT // CHUNK  # 4 chunks
MMF = 512  # max matmul free dim (one PSUM bank of fp32 output)
KD = 24  # dist contraction depth: 4 aug rows x 6 bf16 level-pairs
EPS = 1e-8

_cache = {}


def _build_program(exp_scale: float):
    """Build the single-core Bass program (shared SPMD across all 8 cores)."""
    import concourse.bass as bass
    import concourse.bacc as bacc
    import concourse.tile as tile
    from concourse import mybir
    from contextlib import ExitStack

    f32 = mybir.dt.float32
    bf16 = mybir.dt.bfloat16

    nc = bacc.Bacc("TRN2", target_bir_lowering=False, debug=False)
    d_augx = nc.declare_dram_parameter("aug_x", [KD, N_IN], bf16, isOutput=False)
    d_augt = nc.declare_dram_parameter("aug_t", [KD, N_OUT], bf16, isOutput=False)
    d_dy = nc.declare_dram_parameter("dy", [N_IN, 2], bf16, isOutput=False)
    d_w3 = nc.declare_dram_parameter("w3", [3, OUT_CH], bf16, isOutput=False)
    d_out = nc.declare_dram_parameter("out", [N_OUT, OUT_CH], f32, isOutput=True)

    with ExitStack() as ctx:
        tc = ctx.enter_context(tile.TileContext(nc))
        singles = ctx.enter_context(tc.tile_pool(name="singles", bufs=1))
        wts = ctx.enter_context(tc.tile_pool(name="wts", bufs=4))
        small = ctx.enter_context(tc.tile_pool(name="small", bufs=2))
        outs = ctx.enter_context(tc.tile_pool(name="outs", bufs=4))
        pd = ctx.enter_context(tc.tile_pool(name="pd", bufs=2, space="PSUM"))
        pa = ctx.enter_context(tc.tile_pool(name="pa", bufs=1, space="PSUM"))
        pp = ctx.enter_context(tc.tile_pool(name="pp", bufs=2, space="PSUM"))

        # ---- constants into SBUF ----
        sb_augx = singles.tile([KD, N_IN], bf16)
        nc.sync.dma_start(out=sb_augx, in_=d_augx[:])
        sb_augt = singles.tile([KD, N_OUT], bf16)
        nc.sync.dma_start(out=sb_augt, in_=d_augt[:])
        # dy tiled: n = nt*128 + p  ->  [p, nt, c]
        sb_dy = singles.tile([P, NT, 2], bf16)
        nc.sync.dma_start(out=sb_dy, in_=d_dy.rearrange("(t p) c -> p t c", p=P))
        sb_w3 = singles.tile([3, OUT_CH], bf16)
        nc.sync.dma_start(out=sb_w3, in_=d_w3[:])
        # fp32 staging rows for the divide: 0 = dens, 1 = conv
        sb_feat = singles.tile([2, N_OUT], f32)
        # bf16 projection lhsT rows: 0 = dens, 1 = conv/dens, 2 = 1
        # (compute engines can't address partition base 2, so DMA the ones row
        # from aug_t row 2, which is all-ones by construction)
        sb_featb = singles.tile([3, N_OUT], bf16)
        nc.sync.dma_start(out=sb_featb[2:3, :], in_=d_augt[2:3, :])

        def emit_dist(ch, nt):
            m0 = ch * CHUNK
            dist = pd.tile([P, CHUNK], f32, tag="dist")
            for h in range(CHUNK // MMF):
                nc.tensor.matmul(
                    dist[:, h * MMF : (h + 1) * MMF],
                    sb_augx[:, nt * P : (nt + 1) * P],
                    sb_augt[:, m0 + h * MMF : m0 + (h + 1) * MMF],
                    start=True,
                    stop=True,
                )
            wt = wts.tile([P, CHUNK], bf16, tag="wt")
            nc.scalar.activation(
                wt, dist, mybir.ActivationFunctionType.Exp,
                scale=float(exp_scale),
            )
            return wt

        def emit_reduce(acc, nt, wt):
            for h in range(CHUNK // MMF):
                nc.tensor.matmul(
                    acc[:, h * MMF : (h + 1) * MMF],
                    sb_dy[:, nt, :],
                    wt[:, h * MMF : (h + 1) * MMF],
                    start=(nt == 0),
                    stop=(nt == NT - 1),
                )

        def emit_chunk_tail(ch, acc):
            m0 = ch * CHUNK
            # evacuate [dens; conv] into the fp32 staging rows
            nc.vector.tensor_copy(sb_feat[0:2, m0 : m0 + CHUNK], acc)

            # repack dens/conv to [128, x] so the divide uses all lanes:
            # packed[p, c, f] = feat[c, m0 + p*(CHUNK/P) + f]
            FPP = CHUNK // P  # elements per partition (8)
            packed = small.tile([P, 2, FPP], f32, tag="packed")
            for c in range(2):
                nc.sync.dma_start(
                    out=packed[:, c, :], in_=sb_feat[c : c + 1, m0 : m0 + CHUNK]
                )
            densb = small.tile([P, FPP], bf16, tag="densb")
            nc.vector.tensor_copy(densb, packed[:, 0, :])
            nc.sync.dma_start(out=sb_featb[0:1, m0 : m0 + CHUNK], in_=densb)
            rec = small.tile([P, FPP], f32, tag="rec")
            nc.vector.tensor_scalar_add(rec, packed[:, 0, :], EPS)
            nc.vector.reciprocal(rec, rec)
            q = small.tile([P, FPP], bf16, tag="q")
            nc.vector.tensor_mul(q, packed[:, 1, :], rec)
            # conv/dens into the bf16 projection row 1
            nc.sync.dma_start(out=sb_featb[1:2, m0 : m0 + CHUNK], in_=q)

            # projection for this chunk: out[m, o] = featb[:, m]^T @ w3
            for mt in range(CHUNK // P):
                mm0 = m0 + mt * P
                po = pp.tile([P, OUT_CH], f32, tag="po")
                nc.tensor.matmul(
                    po,
                    sb_featb[:, mm0 : mm0 + P],
                    sb_w3,
                    start=True,
                    stop=True,
                )
                ob = outs.tile([P, OUT_CH], f32, tag="ob")
                nc.vector.tensor_copy(ob, po)
                nc.sync.dma_start(out=d_out[mm0 : mm0 + P, :], in_=ob)

        # software-pipelined by one stage: the PE queue is strict FIFO, so
        # dist(nt+1) must be enqueued BEFORE reduce(nt) -- otherwise the PE
        # sits at reduce(nt) waiting for exp(nt) every iteration (and the HAM
        # clock gate drops the PE back to 1.2 GHz).
        steps = [(ch, nt) for ch in range(NCH) for nt in range(NT)]
        accs = {}
        wt_prev = emit_dist(*steps[0])
        for i, (ch, nt) in enumerate(steps):
            if ch not in accs:
                acc = pa.tile([2, CHUNK], f32, tag="acc")
                accs[ch] = acc
            wt_next = emit_dist(*steps[i + 1]) if i + 1 < len(steps) else None
            emit_reduce(accs[ch], nt, wt_prev)
            wt_prev = wt_next
            if nt == NT - 1:
                emit_chunk_tail(ch, accs.pop(ch))

    nc.compile()
    return nc


def _bf(v):
    """Round fp64/fp32 array to bf16, returned as fp64 for residual math."""
    return np.asarray(v, np.float32).astype(BF16).astype(np.float64)


def _split3_bf16(a64):
    """fp64 -> three bf16 levels, a0+a1+a2 ~= a to ~2^-24."""
    a0 = _bf(a64)
    a1 = _bf(a64 - a0)
    a2 = _bf(a64 - a0 - a1)
    return a0, a1, a2


def _prep_inputs(x, y, t, sigma, W, b):
    """Host-side packing of the augmented operands (numpy, cheap)."""
    x = np.asarray(x, np.float32)
    y = np.asarray(y, np.float32)
    t = np.asarray(t, np.float32)
    sigma = np.asarray(sigma, np.float32)
    W = np.asarray(W, np.float32)
    b = np.asarray(b, np.float32)

    Bb, n_in, _ = x.shape
    n_out = t.shape[1]
    assert (Bb, n_in, n_out) == (B, N_IN, N_OUT), (Bb, n_in, n_out)

    ax64 = np.empty((B, 4, N_IN), np.float64)
    ax64[:, 0] = x[:, :, 0]
    ax64[:, 1] = x[:, :, 1]
    ax64[:, 2] = x[:, :, 0].astype(np.float64) ** 2 + x[:, :, 1].astype(np.float64) ** 2
    ax64[:, 3] = 1.0
    at64 = np.empty((B, 4, N_OUT), np.float64)
    at64[:, 0] = -2.0 * t[:, :, 0].astype(np.float64)
    at64[:, 1] = -2.0 * t[:, :, 1].astype(np.float64)
    at64[:, 2] = 1.0
    at64[:, 3] = t[:, :, 0].astype(np.float64) ** 2 + t[:, :, 1].astype(np.float64) ** 2

    xa = _split3_bf16(ax64)
    ta = _split3_bf16(at64)
    pairs = [(0, 0), (0, 1), (1, 0), (0, 2), (1, 1), (2, 0)]
    aug_x = np.concatenate([xa[i] for i, j in pairs], axis=1).astype(BF16)
    aug_t = np.concatenate([ta[j] for i, j in pairs], axis=1).astype(BF16)

    dy = np.empty((B, N_IN, 2), np.float32)
    dy[:, :, 0] = 1.0
    dy[:, :, 1] = y[:, :, 0]
    dy = dy.astype(BF16)

    w3 = np.empty((3, OUT_CH), np.float32)
    w3[0] = W[:, 0]
    w3[1] = W[:, 1]
    w3[2] = b
    w3 = w3.astype(BF16)

    scales = np.exp(sigma.astype(np.float32))
    exp_scale = (-0.5 / (scales.astype(np.float32) ** 2)).astype(np.float32)
    assert float(exp_scale[0]) == float(exp_scale[1]), "shared-scale kernel"
    return aug_x, aug_t, dy, w3, float(exp_scale[0])


def _run(x, y, t, sigma, W, b, trace):
    from concourse.bass_utils import run_bass_kernel_spmd

    aug_x, aug_t, dy, w3, es = _prep_inputs(x, y, t, sigma, W, b)

    key = es
    if key not in _cache:
        _cache[key] = _build_program(es)
    nc = _cache[key]

    in_maps = [
        {"aug_x": aug_x[i], "aug_t": aug_t[i], "dy": dy[i], "w3": w3}
        for i in range(B)
    ]
    res = run_bass_kernel_spmd(nc, in_maps, list(range(B)), trace=trace)
    out = np.stack([res.results[i]["out"] for i in range(B)])
    return out.astype(np.float32), res.exec_time_ns


def kernel(x, y, t, sigma, W, b, _mm_dtype="bf16"):
    out, _ = _run(x, y, t, sigma, W, b, trace=False)
    return out


def bench(x, y, t, sigma, W, b, _mm_dtype="bf16"):
    """Correctness + HW timing helper (used by test.py, not by the grader)."""
    return _run(x, y, t, sigma, W, b, trace=True)
